# revision 27
# baseline (speedup 1.0000x reference)
from contextlib import ExitStack

import numpy as np

import concourse.bass as bass
import concourse.mybir as mybir
import concourse.tile as tile
from concourse import bacc
from concourse.masks import make_identity

F32 = mybir.dt.float32
F32R = mybir.dt.float32r
F16 = mybir.dt.float16
Exp = mybir.ActivationFunctionType.Exp
Identity = mybir.ActivationFunctionType.Identity
Copy = mybir.ActivationFunctionType.Copy

USE_F32R = True

D = 1024
T = 2048
BATCH = 4
NH = 16
DH = 64
HLOC = 8
DSH = 512
N_CORES = 8

TC = T // 512
KC = T // 128
DC = D // 128


def _build(ablate=()):
    nc = bacc.Bacc("TRN2", target_bir_lowering=False, debug=False,
                   num_devices=N_CORES)
    xT = nc.dram_tensor("xT", [D, T], F32, kind="ExternalInput").ap()
    wqT = nc.dram_tensor("wqT", [D, DSH], F32, kind="ExternalInput").ap()
    wkT = nc.dram_tensor("wkT", [D, DSH], F32, kind="ExternalInput").ap()
    wvT = nc.dram_tensor("wvT", [D, DSH], F32, kind="ExternalInput").ap()
    woT = nc.dram_tensor("woT", [DSH, D], F32, kind="ExternalInput").ap()
    bq = nc.dram_tensor("bq", [DSH], F32, kind="ExternalInput").ap()
    bk = nc.dram_tensor("bk", [DSH], F32, kind="ExternalInput").ap()
    y = nc.dram_tensor("y", [T, D], F32, kind="ExternalOutput").ap()

    FMM = F32R if USE_F32R else F32

    with tile.TileContext(nc) as tc, ExitStack() as ctx:
        singles = ctx.enter_context(tc.tile_pool(name="singles", bufs=1))
        wpool = ctx.enter_context(tc.tile_pool(name="wpool", bufs=1))
        xtpool = ctx.enter_context(tc.tile_pool(name="xtpool", bufs=1))
        tmp_pool = ctx.enter_context(tc.tile_pool(name="tmp", bufs=3))
        qtpool = ctx.enter_context(tc.tile_pool(name="qt", bufs=2))
        attnp = ctx.enter_context(tc.tile_pool(name="attnp", bufs=1))
        attnTp = ctx.enter_context(tc.tile_pool(name="attnTp", bufs=1))
        exp_pool = ctx.enter_context(tc.tile_pool(name="exp", bufs=4))
        small = ctx.enter_context(tc.tile_pool(name="small", bufs=4))
        ybuf = ctx.enter_context(tc.tile_pool(name="ybuf", bufs=2))
        ps_s = ctx.enter_context(tc.tile_pool(name="ps_s", bufs=2, space="PSUM"))
        ps_pv = ctx.enter_context(tc.tile_pool(name="ps_pv", bufs=1, space="PSUM"))
        ps_aux = ctx.enter_context(tc.tile_pool(name="ps_aux", bufs=2, space="PSUM"))

        KT_t = singles.tile([128, 4, T], FMM)
        Vp_t = singles.tile([128, KC, HLOC, DH + 1], F16)
        ident_t = singles.tile([128, 128], F32)
        mask_t = singles.tile([128, 128], F32)
        bq_t = singles.tile([128, 4], F32)
        bk_t = singles.tile([128, 4], F32)

        make_identity(nc, ident_t)
        nc.vector.memset(Vp_t[:, :, :, DH:DH + 1], 1.0)
        nc.gpsimd.memset(mask_t, 1.0)
        nc.gpsimd.affine_select(
            out=mask_t, in_=mask_t,
            compare_op=mybir.AluOpType.is_ge,
            fill=0.0,
            base=0,
            pattern=[[1, 128]],
            channel_multiplier=-1,
        )
        nc.sync.dma_start(out=bq_t, in_=bq.rearrange("(c p) -> p c", p=128))
        nc.sync.dma_start(out=bk_t, in_=bk.rearrange("(c p) -> p c", p=128))

        def load(dst, src):
            if dst.dtype == F32R:
                stage = tmp_pool.tile([128, 512], F32, tag="stage", name="stage")
                nc.sync.dma_start(out=stage, in_=src)
                nc.gpsimd.tensor_copy(dst, stage)
            else:
                nc.sync.dma_start(out=dst, in_=src)

        wq_t = wpool.tile([128, DC, DSH], FMM)
        wk_t = wpool.tile([128, DC, DSH], FMM)
        wv_t = wpool.tile([128, DC, DSH], FMM)
        wo_t = wpool.tile([128, 4, D], FMM)
        wqT_r = wqT.rearrange("(d p) j -> p d j", p=128)
        wkT_r = wkT.rearrange("(d p) j -> p d j", p=128)
        wvT_r = wvT.rearrange("(d p) j -> p d j", p=128)
        woT_r = woT.rearrange("(c p) j -> p c j", p=128)
        for d in range(DC):
            load(wq_t[:, d, :], wqT_r[:, d, :])
            load(wk_t[:, d, :], wkT_r[:, d, :])
            load(wv_t[:, d, :], wvT_r[:, d, :])
        for c in range(4):
            for jc in range(2):
                load(wo_t[:, c, 512 * jc:512 * (jc + 1)],
                     woT_r[:, c, 512 * jc:512 * (jc + 1)])

        xT_r = xT.rearrange("(d p) t -> p d t", p=128)

        for w in range(TC):
            xt = xtpool.tile([128, DC, 512], FMM)
            for d in range(DC):
                load(xt[:, d, :], xT_r[:, d, 512 * w:512 * (w + 1)])
            qt_w = qtpool.tile([128, 4, 512], FMM, tag="qt")
            for c in range(4):
                psp = ps_aux.tile([128, 512], F32, tag="aux", name="psq")
                for d in range(DC):
                    nc.tensor.matmul(
                        psp,
                        lhsT=wq_t[:, d, 128 * c:128 * (c + 1)],
                        rhs=xt[:, d, :],
                        start=(d == 0), stop=(d == DC - 1),
                    )
                nc.vector.tensor_scalar_add(qt_w[:, c, :], psp, bq_t[:, c:c + 1])
                psk = ps_aux.tile([128, 512], F32, tag="aux", name="psk")
                for d in range(DC):
                    nc.tensor.matmul(
                        psk,
                        lhsT=wk_t[:, d, 128 * c:128 * (c + 1)],
                        rhs=xt[:, d, :],
                        start=(d == 0), stop=(d == DC - 1),
                    )
                nc.vector.tensor_scalar_add(
                    KT_t[:, c, 512 * w:512 * (w + 1)], psk, bk_t[:, c:c + 1])
            for s in range(4):
                psv = ps_aux.tile([128, 512], F32, tag="aux", name="psv")
                for d in range(DC):
                    nc.tensor.matmul(
                        psv,
                        lhsT=xt[:, d, 128 * s:128 * (s + 1)],
                        rhs=wv_t[:, d, :],
                        start=(d == 0), stop=(d == DC - 1),
                    )
                nc.vector.tensor_copy(
                    Vp_t[:, 4 * w + s, :, 0:DH],
                    psv.rearrange("p (h v) -> p h v", h=HLOC),
                )

            kmax = 4 * (w + 1)
            attn_t = attnp.tile([128, 4, DSH], F32, tag="attn")
            for h in range(HLOC):
                ch, po = h // 2, (h % 2) * 64
                pso_big = ps_pv.tile([128, 4, 512], F32, tag="pso")
                pso = [pso_big[:, i, 0:DH + 1] for i in range(4)]
                for j in range(kmax):
                    rel = j - 4 * w
                    q0 = max(rel, 0) * 128
                    pss = ps_s.tile([128, 512], F32, tag="pss")
                    if "scores" not in ablate:
                        nc.tensor.matmul(
                            pss[:, q0:],
                            lhsT=KT_t[po:po + 64, ch, 128 * j:128 * (j + 1)],
                            rhs=qt_w[po:po + 64, ch, q0:],
                            start=True, stop=True,
                        )
                    ex = exp_pool.tile([128, 512], F16, tag="ex")
                    if "exp" not in ablate:
                        nc.scalar.activation(out=ex[:, q0:], in_=pss[:, q0:],
                                             func=Exp, scale=0.125)
                    if rel >= 0 and "mask" not in ablate:
                        nc.vector.tensor_mul(
                            ex[:, q0:q0 + 128], ex[:, q0:q0 + 128],
                            mask_t)
                    if "pv" not in ablate:
                        for i in range(max(rel, 0), 4):
                            nc.tensor.matmul(
                                pso[i],
                                lhsT=ex[:, 128 * i:128 * (i + 1)],
                                rhs=Vp_t[:, j, h, :],
                                start=(j == 0), stop=(j == 4 * w + i),
                            )
                if "rescale" not in ablate:
                    rec = small.tile([128, 4, 1], F32, tag="rec")
                    nc.vector.reciprocal(rec, pso_big[:, :, DH:DH + 1])
                    nc.vector.tensor_mul(
                        attn_t[:, :, DH * h:DH * (h + 1)],
                        pso_big[:, :, 0:DH],
                        rec.broadcast_to([128, 4, DH]),
                    )

            if "tail" in ablate:
                continue
            attnT_t = attnTp.tile([128, 4, 512], FMM, tag="attnT")
            for c in range(4):
                pst = ps_aux.tile([128, 512], F32, tag="aux", name="pst")
                for i in range(4):
                    nc.tensor.transpose(
                        pst[:, 128 * i:128 * (i + 1)],
                        attn_t[:, i, 128 * c:128 * (c + 1)], ident_t)
                nc.vector.tensor_copy(attnT_t[:, c, :], pst)
            for i in range(4):
                for jc in range(2):
                    py = ps_aux.tile([128, 512], F32, tag="aux", name="py")
                    for c in range(4):
                        nc.tensor.matmul(
                            py,
                            lhsT=attnT_t[:, c, 128 * i:128 * (i + 1)],
                            rhs=wo_t[:, c, 512 * jc:512 * (jc + 1)],
                            start=(c == 0), stop=(c == 3),
                        )
                    ysb = ybuf.tile([128, 512], F32, tag="ysb")
                    nc.vector.tensor_copy(ysb, py)
                    nc.sync.dma_start(
                        out=y[512 * w + 128 * i:512 * w + 128 * (i + 1),
                              512 * jc:512 * (jc + 1)],
                        in_=ysb,
                    )
    nc.compile()
    return nc


def shard_inputs(x, Wq, bq, Wk, bk, Wv, bv, Wo, bo):
    in_maps = []
    for c in range(N_CORES):
        b, g = c // 2, c % 2
        sl = slice(DSH * g, DSH * (g + 1))
        in_maps.append({
            "xT": np.ascontiguousarray(x[b].T),
            "wqT": np.ascontiguousarray(Wq[sl, :].T),
            "wkT": np.ascontiguousarray(Wk[sl, :].T),
            "wvT": np.ascontiguousarray(Wv[sl, :].T),
            "woT": np.ascontiguousarray(Wo.T[sl, :]),
            "bq": np.ascontiguousarray(bq[sl]),
            "bk": np.ascontiguousarray(bk[sl]),
        })
    return in_maps


def combine_outputs(results, bv, Wo, bo):
    corr = (bv @ Wo.T + bo).astype(np.float32)
    y = np.empty((BATCH, T, D), dtype=np.float32)
    for b in range(BATCH):
        y[b] = results[2 * b]["y"] + results[2 * b + 1]["y"] + corr
    return y


def run_sharded(inputs, trace=False):
    from concourse import bass_utils

    inputs = {k: np.asarray(v, dtype=np.float32) for k, v in inputs.items()}
    nc = _build()
    in_maps = shard_inputs(
        inputs["x"], inputs["Wq"], inputs["bq"], inputs["Wk"], inputs["bk"],
        inputs["Wv"], inputs["bv"], inputs["Wo"], inputs["bo"])
    res = bass_utils.run_bass_kernel_spmd(
        nc, in_maps, list(range(N_CORES)), trace=trace)
    y = combine_outputs(res.results, inputs["bv"], inputs["Wo"], inputs["bo"])
    return y, res


def kernel(**inputs):
    y, _ = run_sharded(inputs, trace=False)
    return y


if __name__ == "__main__":
    rng = np.random.default_rng(0)
    demo = {
        "x": rng.standard_normal((BATCH, T, D), dtype=np.float32),
        "Wq": rng.standard_normal((D, D), dtype=np.float32) * 0.02,
        "bq": np.zeros(D, np.float32),
        "Wk": rng.standard_normal((D, D), dtype=np.float32) * 0.02,
        "bk": np.zeros(D, np.float32),
        "Wv": rng.standard_normal((D, D), dtype=np.float32) * 0.02,
        "bv": np.zeros(D, np.float32),
        "Wo": rng.standard_normal((D, D), dtype=np.float32) * 0.02,
        "bo": np.zeros(D, np.float32),
    }
    out = kernel(**demo)
    print(out.shape, out.dtype)
